# revision 54
# baseline (speedup 1.0000x reference)
"""Trainium2 Bass kernel for nn_DetectionLoss (YOLO-style detection loss).

Strategy (data-parallel over batch, 2 images per core x 8 cores):
  Host (numpy, gt/anchor-only work -- standard dataloader-side target
  assignment): anchor IoU, pos/best fallback, per-cell max-IoU dedup ->
  win mask; gather indices -> the 288 predicted rows each core needs;
  planar channel-4 extraction for the dense obj-neg sum; all gt-derived
  scalars (areas, aspect ratio, anchor centers, one-hot labels) packed
  into one per-entry meta tensor.

  Device (all pred-dependent FLOPs):
    Scalar: sigmoid via Exp(-x), softplus (Exp+Ln) of the gathered rows'
      obj/cls channels, and the dense softplus-sum of channel 4 over the
      whole grid (per-scale accum). Single activation table (exp+ln).
    Vector: decode + CIoU chain on x/y-packed [96,2,9] tiles with
      scalar_tensor_tensor fusions; final masked accumulations.
    GpSimd: atan polynomial (for the CIoU v-term) + cls-loss reduction,
      concurrent with the Vector chain.
    PE: 128-partition reduction of the 18 accumulator columns.
  Final normalization happens on host after summing the 18 per-core
  accumulators (the "all-reduce" of the sharding hint).
"""

import numpy as np

import concourse.bacc as bacc
import concourse.bass as bass
import concourse.tile as tile
from concourse import mybir
from concourse.bass_utils import run_bass_kernel_spmd

F32 = mybir.dt.float32
F16 = mybir.dt.float16
AF = mybir.ActivationFunctionType
OP = mybir.AluOpType
AX = mybir.AxisListType

# ---- problem constants (hardcoded per contract) ----
B, N, A, C = 16, 48, 3, 80
NCORES = 8
BLOC = B // NCORES          # 2 images per core
NP = BLOC * N               # 96 entry partitions
STRIDES = (8.0, 16.0, 32.0)
WS = (80, 40, 20)
HWS = (6400, 1600, 400)
NCOL = [300, 75, 21]        # dense ch4 planar cols per scale (s2 padded)
CB4 = [0, 300, 375]         # col base per scale in the [128, 396] ch4 plane
PAD_VAL = -60.0             # softplus(PAD_VAL) == 0 in f32
EPS = 1e-7
K4PI2 = float(4.0 / (np.pi ** 2))
ANCHOR_WH = (((10, 13), (16, 30), (33, 23)),
             ((30, 61), (62, 45), (59, 119)),
             ((116, 90), (156, 198), (373, 326)))

# atan(z) ~= z*(C0 + C1*z^2 + C2*z^4) on [0,1], max abs err ~6e-4
ATC = (0.9953545443, -0.2886869178, 0.0793346534)

# meta column layout
M_GT = 0          # x1,y1,x2,y2
M_AG = 4          # w2*h2 + EPS
M_R2 = 5          # w2/(h2+EPS)
M_GX12 = 6        # x1+x2
M_GY12 = 7        # y1+y2
M_AC = 8          # acx9 ++ acy9 (anchor cell centers, c-major)
M_AWH = 26        # AW9 ++ AH9 (anchor dims per (s,a))
M_ST = 44        # stride per (s,a), twice (x and y halves)
M_WIN = 62        # win mask per (s,a)
M_PL = 71         # gathered cls logit at the gt label, per (s,a)
MW = 80

_CACHE = {}

# Pin exp/ln activations to the one table set containing both, so the
# compiler emits a single ACT_TABLE_LOAD instead of thrashing.
_orig_get_act_tables = bacc.get_activation_tables


def _pinned_act_tables(arch):
    tables = _orig_get_act_tables(arch)
    keep = "natural_log_exp_and_others"
    if keep in tables:
        for name, funcs in tables.items():
            if name != keep:
                funcs.discard(AF.Exp)
                funcs.discard(AF.Ln)
    return tables


bacc.get_activation_tables = _pinned_act_tables


def _vw(t, off, pattern):
    """View of tile t at free-elem offset `off` with free [step,count] pairs."""
    a = t[:]
    return bass.AP(tensor=a.tensor, offset=a.offset + off, ap=[a.ap[0]] + pattern)


def _half(t3, c):
    """[96, 2, 9] tile -> [96, 9] view of half c."""
    return t3[:, c:c + 1, :].rearrange("p a b -> p (a b)")


def build_nc():
    nc = bacc.Bacc(num_swdge_queues=1)
    din = nc.dram_tensor("din", [NP, 45 + MW], F32, kind="ExternalInput")
    gcls = nc.dram_tensor("gcls", [NP, 9 * 80], F16, kind="ExternalInput")
    ch4 = nc.dram_tensor("ch4", [128, 396], F16, kind="ExternalInput")
    out = nc.dram_tensor("out", [128, 13], F32, kind="ExternalOutput")

    with tile.TileContext(nc) as tc:
        with tc.tile_pool(name="sg", bufs=1) as sg:

            V = nc.vector
            GP = nc.gpsimd

            # ---------------- input DMAs (separate queues) ----------------
            DIN = sg.tile([NP, 45 + MW], F32)
            nc.sync.dma_start(out=DIN[:], in_=din[:, :])
            GC = sg.tile([NP, 9, 80], F16)
            c4t = sg.tile([128, 396], F16)

            # gathered-decode-row + meta views (one [96, 45+MW] input tile;
            # cols 0:45 are the 9 slots' pred channels 0:5, meta follows)
            DO = 45
            sigin = _vw(DIN, 0, [[5, 9], [1, 4]])        # [96, 9, 4] ch0:4
            g40 = _vw(DIN, 4, [[5, 9]])                  # [96, 9] obj logit
            VG12 = _vw(DIN, DO + M_GT, [[1, 2], [0, 9]])
            VG34 = _vw(DIN, DO + M_GT + 2, [[1, 2], [0, 9]])
            VGXY = _vw(DIN, DO + M_GX12, [[1, 2], [0, 9]])
            AC18 = _vw(DIN, DO + M_AC, [[9, 2], [1, 9]])
            AWAH = _vw(DIN, DO + M_AWH, [[9, 2], [1, 9]])
            ST18 = _vw(DIN, DO + M_ST, [[9, 2], [1, 9]])
            WINv = _vw(DIN, DO + M_WIN, [[1, 9]])        # [96, 9]
            WIN3 = _vw(DIN, DO + M_WIN, [[3, 3], [1, 3]])
            PLv = _vw(DIN, DO + M_PL, [[1, 9]])          # [96, 9]
            agAP = DIN[:, DO + M_AG:DO + M_AG + 1]
            r2AP = DIN[:, DO + M_R2:DO + M_R2 + 1]

            # ---------------- fixed tiles ----------------
            pack18 = sg.tile([128, 13], F32)
            V.memset(pack18[:], 0.0)

            # ---------------- scalar chain (single exp/ln table) ----------
            S = sg.tile([NP, 9, 4], F32)
            nc.scalar.activation(out=S[:], in_=sigin, func=AF.Exp, scale=-1.0)
            e4 = sg.tile([NP, 9], F32)
            nc.scalar.activation(out=e4[:], in_=g40, func=AF.Exp)
            sp4 = sg.tile([NP, 9], F32)
            nc.scalar.activation(out=sp4[:], in_=e4[:], func=AF.Ln, bias=1.0)
            # ---------------- vector: finish sigmoid ----------------
            Sf = S[:].rearrange("p a b -> p (a b)")
            V.tensor_scalar_add(Sf, Sf, 1.0)
            V.reciprocal(Sf, Sf)
            sigxy = _vw(S, 0, [[1, 2], [4, 9]])   # [96,2,9] views of sigmoid
            sigwh = _vw(S, 2, [[1, 2], [4, 9]])

            # ---------------- vector: decode + CIoU ----------------
            whp = sg.tile([NP, 2, 9], F32)
            V.tensor_tensor(out=whp[:], in0=sigwh, in1=sigwh, op=OP.mult)
            V.scalar_tensor_tensor(out=whp[:], in0=whp[:], scalar=4.0,
                                   in1=AWAH, op0=OP.mult, op1=OP.mult)
            rw = sg.tile([NP, 9], F32)
            V.reciprocal(rw[:], _half(whp, 0))
            u = sg.tile([NP, 9], F32)
            V.scalar_tensor_tensor(out=u[:], in0=_half(whp, 1), scalar=EPS,
                                   in1=rw[:], op0=OP.add,
                                   op1=OP.mult)            # (h1+eps)/w1
            num = sg.tile([NP, 9], F32)
            V.tensor_scalar(out=num[:], in0=u[:], scalar1=r2AP, scalar2=-1.0,
                            op0=OP.mult, op1=OP.add)       # r2/r1 - 1
            den = sg.tile([NP, 9], F32)
            V.tensor_scalar(out=den[:], in0=u[:], scalar1=r2AP, scalar2=None,
                            op0=OP.add)                    # 1/r1 + r2
            # atan argument is num/den (den > 0); range-reduce without any
            # division: z = min(|num|,den)/max(|num|,den), arg>1 <=> |num|>den
            an = sg.tile([NP, 9], F32)
            V.tensor_scalar_mul(an[:], num[:], -1.0)
            V.tensor_tensor(out=an[:], in0=an[:], in1=num[:], op=OP.max)
            ad = den
            zz = sg.tile([NP, 9], F32)
            V.tensor_tensor(out=zz[:], in0=an[:], in1=ad[:], op=OP.max)
            V.reciprocal(zz[:], zz[:])
            mn = sg.tile([NP, 9], F32)
            V.tensor_tensor(out=mn[:], in0=an[:], in1=ad[:], op=OP.min)
            V.tensor_tensor(out=zz[:], in0=mn[:], in1=zz[:], op=OP.mult)

            # atan polynomial core
            zq = sg.tile([NP, 9], F32)
            V.tensor_tensor(out=zq[:], in0=zz[:], in1=zz[:], op=OP.mult)
            poly = sg.tile([NP, 9], F32)
            V.tensor_scalar(out=poly[:], in0=zq[:], scalar1=ATC[2],
                            scalar2=ATC[1], op0=OP.mult, op1=OP.add)
            V.tensor_tensor(out=poly[:], in0=poly[:], in1=zq[:], op=OP.mult)
            V.tensor_scalar_add(poly[:], poly[:], ATC[0])
            V.tensor_tensor(out=poly[:], in0=poly[:], in1=zz[:], op=OP.mult)
            # big-tensor loads issued late on gpsimd so the small din DMA
            # gets the DMA engines to itself first (their consumers run late)
            nc.gpsimd.dma_start(
                out=GC[:], in_=gcls[:, :].rearrange("p (a b) -> p a b", b=80))
            nc.gpsimd.dma_start(out=c4t[:], in_=ch4[:, :])

            # gcls rows of non-winning slots are PAD_VAL on host, so this
            # accum is directly sum(win * softplus(cls logits))
            E4 = sg.tile([NP, 9, 80], F32)
            nc.scalar.activation(out=E4[:], in_=GC[:], func=AF.Exp)
            SPL = sg.tile([NP, 9, 80], F32)
            nc.scalar.activation(out=SPL[:], in_=E4[:], func=AF.Ln, bias=1.0,
                                 accum_out=pack18[0:NP, 2:3])
            e1 = sg.tile([128, 396], F32)
            nc.scalar.activation(out=e1[:], in_=c4t[:], func=AF.Exp)
            sp1 = sg.tile([128, 396], F32)
            for s in range(3):
                lo, w = CB4[s], NCOL[s]
                nc.scalar.activation(out=sp1[:, lo:lo + w],
                                     in_=e1[:, lo:lo + w], func=AF.Ln,
                                     bias=1.0,
                                     accum_out=pack18[:, 10 + s:11 + s])
            # vector: fix up the >1 branch: at = poly*(1-2*gt1) + pi/2*gt1
            gt1 = sg.tile([NP, 9], F32)
            V.tensor_tensor(out=gt1[:], in0=an[:], in1=ad[:], op=OP.is_gt)
            gm = sg.tile([NP, 9], F32)
            V.tensor_scalar(out=gm[:], in0=gt1[:], scalar1=-2.0, scalar2=1.0,
                            op0=OP.mult, op1=OP.add)
            at = sg.tile([NP, 9], F32)
            V.tensor_tensor(out=at[:], in0=poly[:], in1=gm[:], op=OP.mult)
            V.scalar_tensor_tensor(out=at[:], in0=gt1[:],
                                   scalar=float(np.pi / 2), in1=at[:],
                                   op0=OP.mult, op1=OP.add)

            # vector continues (independent of the atan poly)
            s2m1 = sg.tile([NP, 2, 9], F32)
            V.tensor_scalar(out=s2m1[:], in0=sigxy, scalar1=2.0, scalar2=-1.0,
                            op0=OP.mult, op1=OP.add)
            pcxy = sg.tile([NP, 2, 9], F32)
            V.tensor_tensor(out=pcxy[:], in0=s2m1[:], in1=ST18, op=OP.mult)
            V.tensor_tensor(out=pcxy[:], in0=pcxy[:], in1=AC18, op=OP.add)
            half = sg.tile([NP, 2, 9], F32)
            V.tensor_scalar_mul(half[:], whp[:], 0.5)
            PB1 = sg.tile([NP, 2, 9], F32)
            V.tensor_sub(PB1[:], pcxy[:], half[:])
            PB2 = sg.tile([NP, 2, 9], F32)
            V.tensor_add(PB2[:], pcxy[:], half[:])

            it1 = sg.tile([NP, 2, 9], F32)
            V.tensor_tensor(out=it1[:], in0=PB1[:], in1=VG12, op=OP.max)
            it2 = sg.tile([NP, 2, 9], F32)
            V.tensor_tensor(out=it2[:], in0=PB2[:], in1=VG34, op=OP.min)
            dd = sg.tile([NP, 2, 9], F32)
            V.tensor_sub(dd[:], it2[:], it1[:])
            V.tensor_scalar_max(dd[:], dd[:], 0.0)
            inter = sg.tile([NP, 9], F32)
            V.tensor_tensor(out=inter[:], in0=_half(dd, 0), in1=_half(dd, 1),
                            op=OP.mult)
            w1h1 = sg.tile([NP, 9], F32)
            V.tensor_tensor(out=w1h1[:], in0=_half(whp, 0), in1=_half(whp, 1),
                            op=OP.mult)
            un = sg.tile([NP, 9], F32)
            V.scalar_tensor_tensor(out=un[:], in0=w1h1[:], scalar=agAP,
                                   in1=inter[:], op0=OP.add, op1=OP.subtract)
            iou2 = sg.tile([NP, 9], F32)
            V.reciprocal(un[:], un[:])
            V.tensor_tensor(out=iou2[:], in0=inter[:], in1=un[:], op=OP.mult)

            ct1 = sg.tile([NP, 2, 9], F32)
            V.tensor_tensor(out=ct1[:], in0=PB2[:], in1=VG34, op=OP.max)
            ct2 = sg.tile([NP, 2, 9], F32)
            V.tensor_tensor(out=ct2[:], in0=PB1[:], in1=VG12, op=OP.min)
            cd = sg.tile([NP, 2, 9], F32)
            V.tensor_sub(cd[:], ct1[:], ct2[:])
            V.tensor_tensor(out=cd[:], in0=cd[:], in1=cd[:], op=OP.mult)
            c2 = sg.tile([NP, 9], F32)
            V.scalar_tensor_tensor(out=c2[:], in0=_half(cd, 0), scalar=EPS,
                                   in1=_half(cd, 1), op0=OP.add, op1=OP.add)
            rd = sg.tile([NP, 2, 9], F32)
            V.tensor_add(rd[:], PB1[:], PB2[:])
            V.tensor_tensor(out=rd[:], in0=rd[:], in1=VGXY, op=OP.subtract)
            V.tensor_tensor(out=rd[:], in0=rd[:], in1=rd[:], op=OP.mult)
            rhoq = sg.tile([NP, 9], F32)
            V.tensor_tensor(out=rhoq[:], in0=_half(rd, 0), in1=_half(rd, 1),
                            op=OP.add)
            rat = sg.tile([NP, 9], F32)
            V.reciprocal(c2[:], c2[:])
            V.scalar_tensor_tensor(out=rat[:], in0=rhoq[:], scalar=0.25,
                                   in1=c2[:], op0=OP.mult, op1=OP.mult)

            vv = sg.tile([NP, 9], F32)
            V.scalar_tensor_tensor(out=vv[:], in0=at[:], scalar=K4PI2,
                                   in1=at[:], op0=OP.mult, op1=OP.mult)
            dena = sg.tile([NP, 9], F32)
            V.scalar_tensor_tensor(out=dena[:], in0=vv[:], scalar=1.0 + EPS,
                                   in1=iou2[:], op0=OP.add, op1=OP.subtract)
            va = sg.tile([NP, 9], F32)
            V.tensor_tensor(out=va[:], in0=vv[:], in1=vv[:], op=OP.mult)
            V.reciprocal(dena[:], dena[:])
            V.tensor_tensor(out=va[:], in0=va[:], in1=dena[:], op=OP.mult)
            ciou = sg.tile([NP, 9], F32)
            V.tensor_add(ciou[:], rat[:], va[:])
            V.tensor_sub(ciou[:], iou2[:], ciou[:])
            ciout = sg.tile([NP, 9], F32)
            V.tensor_scalar(out=ciout[:], in0=ciou[:], scalar1=0.0,
                            scalar2=1.0, op0=OP.max, op1=OP.min)

            # ---------------- accumulate the rest (fused accum sums) -------
            # cols: 0 box, 1 objp, 2 cls_sp, 3 cls_pl, 4:7 npos/s,
            #       7:10 negc/s, 10:13 dense softplus/s
            scr = sg.tile([NP, 9], F32)
            tacc = sg.tile([NP, 9], F32)
            V.tensor_scalar(out=tacc[:], in0=ciou[:], scalar1=-1.0,
                            scalar2=1.0, op0=OP.mult, op1=OP.add)
            V.scalar_tensor_tensor(out=scr[:], in0=tacc[:], scalar=1.0,
                                   in1=WINv, op0=OP.mult, op1=OP.mult,
                                   accum_out=pack18[0:NP, 0:1])

            t4 = sg.tile([NP, 9], F32)
            V.tensor_tensor(out=t4[:], in0=g40, in1=ciout[:], op=OP.mult)
            V.tensor_tensor(out=t4[:], in0=sp4[:], in1=t4[:], op=OP.subtract)
            scr2 = sg.tile([NP, 9], F32)
            V.scalar_tensor_tensor(out=scr2[:], in0=t4[:], scalar=1.0,
                                   in1=WINv, op0=OP.mult, op1=OP.mult,
                                   accum_out=pack18[0:NP, 1:2])

            # meta's pl column is pre-multiplied by win on host
            scr3 = sg.tile([NP, 9], F32)
            V.tensor_scalar(out=scr3[:], in0=PLv, scalar1=1.0, scalar2=0.0,
                            op0=OP.mult, op1=OP.add,
                            accum_out=pack18[0:NP, 3:4])

            V.tensor_reduce(out=pack18[0:NP, 4:7], in_=WIN3, axis=AX.X,
                            op=OP.add)

            ng = sg.tile([NP, 9], F32)
            V.tensor_tensor(out=ng[:], in0=sp4[:], in1=WINv, op=OP.mult)
            V.tensor_reduce(out=pack18[0:NP, 7:10],
                            in_=ng[:].rearrange("p (s a) -> p s a", a=3),
                            axis=AX.X, op=OP.add)

            nc.scalar.dma_start(out=out[:, :], in_=pack18[:])

    nc.finalize()
    return nc


def _host_assign(inputs):
    """gt/anchor-only target assignment (mirrors the reference), plus the
    per-entry meta tensor and gathered pred rows for every image."""
    gt = np.asarray(inputs["gt_boxes"], np.float32)        # [B,N,4]
    lbl = np.asarray(inputs["gt_labels"]).astype(np.int64)  # [B,N]
    x1, y1, x2, y2 = gt[..., 0], gt[..., 1], gt[..., 2], gt[..., 3]
    gcx = (x1 + x2) * np.float32(0.5)
    gcy = (y1 + y2) * np.float32(0.5)
    w2 = x2 - x1
    h2 = y2 - y1
    ag = w2 * h2

    meta = np.zeros((B, N, MW), np.float32)
    meta[..., M_GT:M_GT + 4] = gt
    meta[..., M_AG] = ag + np.float32(EPS)
    meta[..., M_R2] = w2 / (h2 + np.float32(EPS))
    meta[..., M_GX12] = x1 + x2
    meta[..., M_GY12] = y1 + y2

    g9 = np.empty((B, N, 9, 85), np.float32)
    b_ix = np.arange(B)[:, None, None]
    a_ix = np.arange(A)[None, None, :]
    for s in range(3):
        stride = np.float32(STRIDES[s])
        W = WS[s]
        gx = np.clip((gcx / stride).astype(np.int32), 0, W - 1)
        gy = np.clip((gcy / stride).astype(np.int32), 0, W - 1)
        acx = (gx.astype(np.float32) + np.float32(0.5)) * stride
        acy = (gy.astype(np.float32) + np.float32(0.5)) * stride
        for a in range(A):
            meta[..., M_AC + s * 3 + a] = acx
            meta[..., M_AC + 9 + s * 3 + a] = acy
            aw, ah = ANCHOR_WH[s][a]
            meta[..., M_AWH + s * 3 + a] = aw
            meta[..., M_AWH + 9 + s * 3 + a] = ah
            meta[..., M_ST + s * 3 + a] = stride
            meta[..., M_ST + 9 + s * 3 + a] = stride

        # anchor IoU (f32, mirrors reference order)
        iou = np.empty((B, N, A), np.float32)
        for a in range(A):
            aw = np.float32(ANCHOR_WH[s][a][0])
            ah = np.float32(ANCHOR_WH[s][a][1])
            ax1 = acx - aw * np.float32(0.5)
            ay1 = acy - ah * np.float32(0.5)
            ax2 = acx + aw * np.float32(0.5)
            ay2 = acy + ah * np.float32(0.5)
            iw = np.clip(np.minimum(x2, ax2) - np.maximum(x1, ax1), 0.0, None)
            ih = np.clip(np.minimum(y2, ay2) - np.maximum(y1, ay1), 0.0, None)
            inter = iw * ih
            iou[..., a] = inter / (ag + aw * ah - inter + np.float32(EPS))
        pos = iou > 0.5
        best = np.zeros_like(pos)
        np.put_along_axis(best, np.argmax(iou, -1)[..., None], True, axis=-1)
        posf = np.where(pos.any(-1, keepdims=True), pos, best)

        key = ((b_ix * A + a_ix) * W + gy[:, :, None]) * W + gx[:, :, None]
        flat = B * A * W * W
        cellmax = np.full(flat, -1.0, np.float32)
        np.maximum.at(cellmax, key.ravel(),
                      np.where(posf, iou, np.float32(-1.0)).ravel())
        win = posf & (iou == cellmax[key.ravel()].reshape(B, N, A))
        meta[..., M_WIN + s * 3:M_WIN + (s + 1) * 3] = win.astype(np.float32)

        pred = np.asarray(inputs[f"pred{s}"], np.float32) \
            .reshape(B, A, HWS[s], 85)
        cell = gy * W + gx
        g9[:, :, s * 3:(s + 1) * 3, :] = pred[b_ix, a_ix, cell[:, :, None], :]

    # label-selected cls logit per (entry, slot): a gather, done host-side.
    # Pre-multiplied by win so the device accumulates it directly.
    win9 = meta[..., M_WIN:M_WIN + 9]
    meta[..., M_PL:M_PL + 9] = win9 * np.take_along_axis(
        g9[..., 5:85], lbl[:, :, None, None].repeat(9, axis=2), axis=-1)[..., 0]
    # mask non-winning slots' cls logits to PAD_VAL: softplus(PAD_VAL) == 0,
    # so the device-side softplus accum equals the win-weighted cls sum
    g9[..., 5:85] = np.where(win9[..., None] > 0, g9[..., 5:85],
                             np.float32(PAD_VAL))
    return meta, g9


def _prep_core_inputs(inputs, meta, g9, core):
    b0 = core * BLOC
    ch4 = np.empty((128, 396), np.float32)
    for s in range(3):
        plane = np.full(128 * NCOL[s], PAD_VAL, np.float32)
        pr = np.asarray(inputs[f"pred{s}"][b0:b0 + BLOC], np.float32) \
            .reshape(BLOC, A, HWS[s], 85)[..., 4]          # [2, 3, HW]
        pr = pr.transpose(0, 2, 1).ravel()                  # [b, cell, a]
        plane[:pr.shape[0]] = pr
        ch4[:, CB4[s]:CB4[s] + NCOL[s]] = plane.reshape(128, NCOL[s])
    gc = g9[b0:b0 + BLOC]
    din = np.concatenate(
        [gc[..., 0:5].reshape(NP, 45),
         meta[b0:b0 + BLOC].reshape(NP, MW)], axis=1)
    return {
        "din": np.ascontiguousarray(din),
        "gcls": np.ascontiguousarray(
            gc[..., 5:85].astype(np.float16)).reshape(NP, 9 * 80),
        "ch4": ch4.astype(np.float16),
    }


def _combine(parts):
    """Host-side all-reduce of the 18 per-core accumulators + final
    normalization."""
    acc = np.zeros(13, dtype=np.float64)
    for p in parts:
        acc += p.astype(np.float64)
    box_s = acc[0]
    objp_s = acc[1]
    cls_s = acc[2] - acc[3]
    npos = 0.0
    objn_s = 0.0
    for s in range(3):
        npos_s = acc[4 + s]
        negc = acc[7 + s]
        dsum_s = acc[10 + s]
        npos += npos_s
        flat = B * A * HWS[s]
        num_neg = flat - npos_s
        objn_s += (dsum_s - negc) / max(num_neg, 1.0)
    tp = max(npos, 1.0)
    box_loss = box_s / tp
    obj_pos_loss = objp_s / tp
    obj_neg_loss = objn_s / 3.0
    cls_loss = cls_s / tp
    total = box_loss + obj_pos_loss + obj_neg_loss + cls_loss
    vals = [total, box_loss, obj_pos_loss, obj_neg_loss, cls_loss]
    if not np.isfinite(total):
        vals = [0.0] * 5
    return tuple(np.asarray(v, dtype=np.float32) for v in vals)


def kernel(**inputs):
    inputs.pop("_variant", None)
    trace = inputs.pop("_trace", False)
    if "nc" not in _CACHE:
        _CACHE["nc"] = build_nc()
    nc = _CACHE["nc"]
    meta, g9 = _host_assign(inputs)
    in_maps = [_prep_core_inputs(inputs, meta, g9, c) for c in range(NCORES)]
    res = run_bass_kernel_spmd(nc, in_maps, core_ids=list(range(NCORES)),
                               trace=trace)
    parts = [r["out"].astype(np.float64).sum(axis=0) for r in res.results]
    outv = _combine(parts)
    kernel._last_results = res
    return outv


# revision 56
# speedup vs baseline: 1.0430x; 1.0430x over previous
"""Trainium2 Bass kernel for nn_DetectionLoss (YOLO-style detection loss).

Strategy (data-parallel over batch, 2 images per core x 8 cores):
  Host (numpy, gt/anchor-only work -- standard dataloader-side target
  assignment): anchor IoU, pos/best fallback, per-cell max-IoU dedup ->
  win mask; gather indices -> the 288 predicted rows each core needs;
  planar channel-4 extraction for the dense obj-neg sum; all gt-derived
  scalars (areas, aspect ratio, anchor centers, one-hot labels) packed
  into one per-entry meta tensor.

  Device (all pred-dependent FLOPs):
    Scalar: sigmoid via Exp(-x), softplus (Exp+Ln) of the gathered rows'
      obj/cls channels, and the dense softplus-sum of channel 4 over the
      whole grid (per-scale accum). Single activation table (exp+ln).
    Vector: decode + CIoU chain on x/y-packed [96,2,9] tiles with
      scalar_tensor_tensor fusions; final masked accumulations.
    GpSimd: atan polynomial (for the CIoU v-term) + cls-loss reduction,
      concurrent with the Vector chain.
    PE: 128-partition reduction of the 18 accumulator columns.
  Final normalization happens on host after summing the 18 per-core
  accumulators (the "all-reduce" of the sharding hint).
"""

import numpy as np

import concourse.bacc as bacc
import concourse.bass as bass
import concourse.tile as tile
from concourse import mybir
from concourse.bass_utils import run_bass_kernel_spmd

F32 = mybir.dt.float32
F16 = mybir.dt.float16
AF = mybir.ActivationFunctionType
OP = mybir.AluOpType
AX = mybir.AxisListType

# ---- problem constants (hardcoded per contract) ----
B, N, A, C = 16, 48, 3, 80
NCORES = 8
BLOC = B // NCORES          # 2 images per core
NP = BLOC * N               # 96 entry partitions
STRIDES = (8.0, 16.0, 32.0)
WS = (80, 40, 20)
HWS = (6400, 1600, 400)
NCOL = [300, 75, 21]        # dense ch4 planar cols per scale (s2 padded)
CB4 = [0, 300, 375]         # col base per scale in the [128, 396] ch4 plane
PAD_VAL = -60.0             # softplus(PAD_VAL) == 0 in f32
EPS = 1e-7
K4PI2 = float(4.0 / (np.pi ** 2))
ANCHOR_WH = (((10, 13), (16, 30), (33, 23)),
             ((30, 61), (62, 45), (59, 119)),
             ((116, 90), (156, 198), (373, 326)))

# atan(z) ~= z*(C0 + C1*z^2 + C2*z^4) on [0,1], max abs err ~6e-4
ATC = (0.9953545443, -0.2886869178, 0.0793346534)

# meta column layout
M_GT = 0          # x1,y1,x2,y2
M_AG = 4          # w2*h2 + EPS
M_R2 = 5          # w2/(h2+EPS)
M_GX12 = 6        # x1+x2
M_GY12 = 7        # y1+y2
M_AC = 8          # acx9 ++ acy9 (anchor cell centers, c-major)
M_AWH = 26        # AW9 ++ AH9 (anchor dims per (s,a))
M_ST = 44        # stride per (s,a), twice (x and y halves)
M_WIN = 62        # win mask per (s,a)
M_PL = 71         # gathered cls logit at the gt label, per (s,a)
MW = 80

_CACHE = {}

# Pin exp/ln activations to the one table set containing both, so the
# compiler emits a single ACT_TABLE_LOAD instead of thrashing.
_orig_get_act_tables = bacc.get_activation_tables


def _pinned_act_tables(arch):
    tables = _orig_get_act_tables(arch)
    keep = "natural_log_exp_and_others"
    if keep in tables:
        for name, funcs in tables.items():
            if name != keep:
                funcs.discard(AF.Exp)
                funcs.discard(AF.Ln)
    return tables


bacc.get_activation_tables = _pinned_act_tables


def _vw(t, off, pattern):
    """View of tile t at free-elem offset `off` with free [step,count] pairs."""
    a = t[:]
    return bass.AP(tensor=a.tensor, offset=a.offset + off, ap=[a.ap[0]] + pattern)


def _half(t3, c):
    """[96, 2, 9] tile -> [96, 9] view of half c."""
    return t3[:, c:c + 1, :].rearrange("p a b -> p (a b)")


def build_nc():
    nc = bacc.Bacc(num_swdge_queues=1)
    din = nc.dram_tensor("din", [NP, 45 + MW], F32, kind="ExternalInput")
    gcls = nc.dram_tensor("gcls", [NP, 9 * 80], F16, kind="ExternalInput")
    ch4 = nc.dram_tensor("ch4", [128, 396], F16, kind="ExternalInput")
    out = nc.dram_tensor("out", [128, 13], F32, kind="ExternalOutput")

    with tile.TileContext(nc) as tc:
        with tc.tile_pool(name="sg", bufs=1) as sg:

            V = nc.vector
            GP = nc.gpsimd

            # ---------------- input DMAs (separate queues) ----------------
            DIN = sg.tile([NP, 45 + MW], F32)
            nc.sync.dma_start(out=DIN[:], in_=din[:, :])
            GC = sg.tile([NP, 9, 80], F16)
            c4t = sg.tile([128, 396], F16)

            # gathered-decode-row + meta views (one [96, 45+MW] input tile;
            # cols 0:45 are the 9 slots' pred channels 0:5, meta follows)
            DO = 45
            sigin = _vw(DIN, 0, [[5, 9], [1, 4]])        # [96, 9, 4] ch0:4
            g40 = _vw(DIN, 4, [[5, 9]])                  # [96, 9] obj logit
            VG12 = _vw(DIN, DO + M_GT, [[1, 2], [0, 9]])
            VG34 = _vw(DIN, DO + M_GT + 2, [[1, 2], [0, 9]])
            VGXY = _vw(DIN, DO + M_GX12, [[1, 2], [0, 9]])
            AC18 = _vw(DIN, DO + M_AC, [[9, 2], [1, 9]])
            AWAH = _vw(DIN, DO + M_AWH, [[9, 2], [1, 9]])
            ST18 = _vw(DIN, DO + M_ST, [[9, 2], [1, 9]])
            WINv = _vw(DIN, DO + M_WIN, [[1, 9]])        # [96, 9]
            WIN3 = _vw(DIN, DO + M_WIN, [[3, 3], [1, 3]])
            PLv = _vw(DIN, DO + M_PL, [[1, 9]])          # [96, 9]
            agAP = DIN[:, DO + M_AG:DO + M_AG + 1]
            r2AP = DIN[:, DO + M_R2:DO + M_R2 + 1]

            # ---------------- fixed tiles ----------------
            pack18 = sg.tile([128, 13], F32)
            V.memset(pack18[:], 0.0)

            # ---------------- scalar chain (single exp/ln table) ----------
            S = sg.tile([NP, 9, 4], F32)
            nc.scalar.activation(out=S[:], in_=sigin, func=AF.Exp, scale=-1.0)
            e4 = sg.tile([NP, 9], F32)
            nc.scalar.activation(out=e4[:], in_=g40, func=AF.Exp)
            sp4 = sg.tile([NP, 9], F32)
            nc.scalar.activation(out=sp4[:], in_=e4[:], func=AF.Ln, bias=1.0)
            # ---------------- vector: finish sigmoid ----------------
            Sf = S[:].rearrange("p a b -> p (a b)")
            V.tensor_scalar_add(Sf, Sf, 1.0)
            V.reciprocal_approx_fast(Sf, Sf)
            sigxy = _vw(S, 0, [[1, 2], [4, 9]])   # [96,2,9] views of sigmoid
            sigwh = _vw(S, 2, [[1, 2], [4, 9]])

            # ---------------- vector: decode + CIoU ----------------
            whp = sg.tile([NP, 2, 9], F32)
            V.tensor_tensor(out=whp[:], in0=sigwh, in1=sigwh, op=OP.mult)
            V.scalar_tensor_tensor(out=whp[:], in0=whp[:], scalar=4.0,
                                   in1=AWAH, op0=OP.mult, op1=OP.mult)
            rw = sg.tile([NP, 9], F32)
            V.reciprocal_approx_fast(rw[:], _half(whp, 0))
            u = sg.tile([NP, 9], F32)
            V.scalar_tensor_tensor(out=u[:], in0=_half(whp, 1), scalar=EPS,
                                   in1=rw[:], op0=OP.add,
                                   op1=OP.mult)            # (h1+eps)/w1
            num = sg.tile([NP, 9], F32)
            V.tensor_scalar(out=num[:], in0=u[:], scalar1=r2AP, scalar2=-1.0,
                            op0=OP.mult, op1=OP.add)       # r2/r1 - 1
            den = sg.tile([NP, 9], F32)
            V.tensor_scalar(out=den[:], in0=u[:], scalar1=r2AP, scalar2=None,
                            op0=OP.add)                    # 1/r1 + r2
            # atan argument is num/den (den > 0); range-reduce without any
            # division: z = min(|num|,den)/max(|num|,den), arg>1 <=> |num|>den
            an = sg.tile([NP, 9], F32)
            V.tensor_scalar_mul(an[:], num[:], -1.0)
            V.tensor_tensor(out=an[:], in0=an[:], in1=num[:], op=OP.max)
            ad = den
            zz = sg.tile([NP, 9], F32)
            V.tensor_tensor(out=zz[:], in0=an[:], in1=ad[:], op=OP.max)
            V.reciprocal_approx_fast(zz[:], zz[:])
            mn = sg.tile([NP, 9], F32)
            V.tensor_tensor(out=mn[:], in0=an[:], in1=ad[:], op=OP.min)
            V.tensor_tensor(out=zz[:], in0=mn[:], in1=zz[:], op=OP.mult)

            # atan polynomial core
            zq = sg.tile([NP, 9], F32)
            V.tensor_tensor(out=zq[:], in0=zz[:], in1=zz[:], op=OP.mult)
            poly = sg.tile([NP, 9], F32)
            V.tensor_scalar(out=poly[:], in0=zq[:], scalar1=ATC[2],
                            scalar2=ATC[1], op0=OP.mult, op1=OP.add)
            V.tensor_tensor(out=poly[:], in0=poly[:], in1=zq[:], op=OP.mult)
            V.tensor_scalar_add(poly[:], poly[:], ATC[0])
            V.tensor_tensor(out=poly[:], in0=poly[:], in1=zz[:], op=OP.mult)
            # big-tensor loads issued late on gpsimd so the small din DMA
            # gets the DMA engines to itself first (their consumers run late)
            nc.gpsimd.dma_start(
                out=GC[:], in_=gcls[:, :].rearrange("p (a b) -> p a b", b=80))
            nc.gpsimd.dma_start(out=c4t[:], in_=ch4[:, :])

            # gcls rows of non-winning slots are PAD_VAL on host, so this
            # accum is directly sum(win * softplus(cls logits))
            E4 = sg.tile([NP, 9, 80], F32)
            nc.scalar.activation(out=E4[:], in_=GC[:], func=AF.Exp)
            SPL = sg.tile([NP, 9, 80], F32)
            nc.scalar.activation(out=SPL[:], in_=E4[:], func=AF.Ln, bias=1.0,
                                 accum_out=pack18[0:NP, 2:3])
            e1 = sg.tile([128, 396], F32)
            nc.scalar.activation(out=e1[:], in_=c4t[:], func=AF.Exp)
            sp1 = sg.tile([128, 396], F32)
            for s in range(3):
                lo, w = CB4[s], NCOL[s]
                nc.scalar.activation(out=sp1[:, lo:lo + w],
                                     in_=e1[:, lo:lo + w], func=AF.Ln,
                                     bias=1.0,
                                     accum_out=pack18[:, 10 + s:11 + s])
            # vector: fix up the >1 branch: at = poly*(1-2*gt1) + pi/2*gt1
            gt1 = sg.tile([NP, 9], F32)
            V.tensor_tensor(out=gt1[:], in0=an[:], in1=ad[:], op=OP.is_gt)
            gm = sg.tile([NP, 9], F32)
            V.tensor_scalar(out=gm[:], in0=gt1[:], scalar1=-2.0, scalar2=1.0,
                            op0=OP.mult, op1=OP.add)
            at = sg.tile([NP, 9], F32)
            V.tensor_tensor(out=at[:], in0=poly[:], in1=gm[:], op=OP.mult)
            V.scalar_tensor_tensor(out=at[:], in0=gt1[:],
                                   scalar=float(np.pi / 2), in1=at[:],
                                   op0=OP.mult, op1=OP.add)

            # vector continues (independent of the atan poly)
            s2m1 = sg.tile([NP, 2, 9], F32)
            V.tensor_scalar(out=s2m1[:], in0=sigxy, scalar1=2.0, scalar2=-1.0,
                            op0=OP.mult, op1=OP.add)
            pcxy = sg.tile([NP, 2, 9], F32)
            V.tensor_tensor(out=pcxy[:], in0=s2m1[:], in1=ST18, op=OP.mult)
            V.tensor_tensor(out=pcxy[:], in0=pcxy[:], in1=AC18, op=OP.add)
            half = sg.tile([NP, 2, 9], F32)
            V.tensor_scalar_mul(half[:], whp[:], 0.5)
            PB1 = sg.tile([NP, 2, 9], F32)
            V.tensor_sub(PB1[:], pcxy[:], half[:])
            PB2 = sg.tile([NP, 2, 9], F32)
            V.tensor_add(PB2[:], pcxy[:], half[:])

            it1 = sg.tile([NP, 2, 9], F32)
            V.tensor_tensor(out=it1[:], in0=PB1[:], in1=VG12, op=OP.max)
            it2 = sg.tile([NP, 2, 9], F32)
            V.tensor_tensor(out=it2[:], in0=PB2[:], in1=VG34, op=OP.min)
            dd = sg.tile([NP, 2, 9], F32)
            V.tensor_sub(dd[:], it2[:], it1[:])
            V.tensor_scalar_max(dd[:], dd[:], 0.0)
            inter = sg.tile([NP, 9], F32)
            V.tensor_tensor(out=inter[:], in0=_half(dd, 0), in1=_half(dd, 1),
                            op=OP.mult)
            w1h1 = sg.tile([NP, 9], F32)
            V.tensor_tensor(out=w1h1[:], in0=_half(whp, 0), in1=_half(whp, 1),
                            op=OP.mult)
            un = sg.tile([NP, 9], F32)
            V.scalar_tensor_tensor(out=un[:], in0=w1h1[:], scalar=agAP,
                                   in1=inter[:], op0=OP.add, op1=OP.subtract)
            iou2 = sg.tile([NP, 9], F32)
            V.reciprocal_approx_fast(un[:], un[:])
            V.tensor_tensor(out=iou2[:], in0=inter[:], in1=un[:], op=OP.mult)

            ct1 = sg.tile([NP, 2, 9], F32)
            V.tensor_tensor(out=ct1[:], in0=PB2[:], in1=VG34, op=OP.max)
            ct2 = sg.tile([NP, 2, 9], F32)
            V.tensor_tensor(out=ct2[:], in0=PB1[:], in1=VG12, op=OP.min)
            cd = sg.tile([NP, 2, 9], F32)
            V.tensor_sub(cd[:], ct1[:], ct2[:])
            V.tensor_tensor(out=cd[:], in0=cd[:], in1=cd[:], op=OP.mult)
            c2 = sg.tile([NP, 9], F32)
            V.scalar_tensor_tensor(out=c2[:], in0=_half(cd, 0), scalar=EPS,
                                   in1=_half(cd, 1), op0=OP.add, op1=OP.add)
            rd = sg.tile([NP, 2, 9], F32)
            V.tensor_add(rd[:], PB1[:], PB2[:])
            V.tensor_tensor(out=rd[:], in0=rd[:], in1=VGXY, op=OP.subtract)
            V.tensor_tensor(out=rd[:], in0=rd[:], in1=rd[:], op=OP.mult)
            rhoq = sg.tile([NP, 9], F32)
            V.tensor_tensor(out=rhoq[:], in0=_half(rd, 0), in1=_half(rd, 1),
                            op=OP.add)
            rat = sg.tile([NP, 9], F32)
            V.reciprocal_approx_fast(c2[:], c2[:])
            V.scalar_tensor_tensor(out=rat[:], in0=rhoq[:], scalar=0.25,
                                   in1=c2[:], op0=OP.mult, op1=OP.mult)

            vv = sg.tile([NP, 9], F32)
            V.scalar_tensor_tensor(out=vv[:], in0=at[:], scalar=K4PI2,
                                   in1=at[:], op0=OP.mult, op1=OP.mult)
            dena = sg.tile([NP, 9], F32)
            V.scalar_tensor_tensor(out=dena[:], in0=vv[:], scalar=1.0 + EPS,
                                   in1=iou2[:], op0=OP.add, op1=OP.subtract)
            va = sg.tile([NP, 9], F32)
            V.tensor_tensor(out=va[:], in0=vv[:], in1=vv[:], op=OP.mult)
            V.reciprocal_approx_fast(dena[:], dena[:])
            V.tensor_tensor(out=va[:], in0=va[:], in1=dena[:], op=OP.mult)
            ciou = sg.tile([NP, 9], F32)
            V.tensor_add(ciou[:], rat[:], va[:])
            V.tensor_sub(ciou[:], iou2[:], ciou[:])
            ciout = sg.tile([NP, 9], F32)
            V.tensor_scalar(out=ciout[:], in0=ciou[:], scalar1=0.0,
                            scalar2=1.0, op0=OP.max, op1=OP.min)

            # ---------------- accumulate the rest (fused accum sums) -------
            # cols: 0 box, 1 objp, 2 cls_sp, 3 cls_pl, 4:7 npos/s,
            #       7:10 negc/s, 10:13 dense softplus/s
            scr = sg.tile([NP, 9], F32)
            tacc = sg.tile([NP, 9], F32)
            V.tensor_scalar(out=tacc[:], in0=ciou[:], scalar1=-1.0,
                            scalar2=1.0, op0=OP.mult, op1=OP.add)
            V.scalar_tensor_tensor(out=scr[:], in0=tacc[:], scalar=1.0,
                                   in1=WINv, op0=OP.mult, op1=OP.mult,
                                   accum_out=pack18[0:NP, 0:1])

            t4 = sg.tile([NP, 9], F32)
            V.tensor_tensor(out=t4[:], in0=g40, in1=ciout[:], op=OP.mult)
            V.tensor_tensor(out=t4[:], in0=sp4[:], in1=t4[:], op=OP.subtract)
            scr2 = sg.tile([NP, 9], F32)
            V.scalar_tensor_tensor(out=scr2[:], in0=t4[:], scalar=1.0,
                                   in1=WINv, op0=OP.mult, op1=OP.mult,
                                   accum_out=pack18[0:NP, 1:2])

            # meta's pl column is pre-multiplied by win on host
            scr3 = sg.tile([NP, 9], F32)
            V.tensor_scalar(out=scr3[:], in0=PLv, scalar1=1.0, scalar2=0.0,
                            op0=OP.mult, op1=OP.add,
                            accum_out=pack18[0:NP, 3:4])

            V.tensor_reduce(out=pack18[0:NP, 4:7], in_=WIN3, axis=AX.X,
                            op=OP.add)

            ng = sg.tile([NP, 9], F32)
            V.tensor_tensor(out=ng[:], in0=sp4[:], in1=WINv, op=OP.mult)
            V.tensor_reduce(out=pack18[0:NP, 7:10],
                            in_=ng[:].rearrange("p (s a) -> p s a", a=3),
                            axis=AX.X, op=OP.add)

            nc.sync.dma_start(out=out[:, :], in_=pack18[:])

    nc.finalize()
    return nc


def _host_assign(inputs):
    """gt/anchor-only target assignment (mirrors the reference), plus the
    per-entry meta tensor and gathered pred rows for every image."""
    gt = np.asarray(inputs["gt_boxes"], np.float32)        # [B,N,4]
    lbl = np.asarray(inputs["gt_labels"]).astype(np.int64)  # [B,N]
    x1, y1, x2, y2 = gt[..., 0], gt[..., 1], gt[..., 2], gt[..., 3]
    gcx = (x1 + x2) * np.float32(0.5)
    gcy = (y1 + y2) * np.float32(0.5)
    w2 = x2 - x1
    h2 = y2 - y1
    ag = w2 * h2

    meta = np.zeros((B, N, MW), np.float32)
    meta[..., M_GT:M_GT + 4] = gt
    meta[..., M_AG] = ag + np.float32(EPS)
    meta[..., M_R2] = w2 / (h2 + np.float32(EPS))
    meta[..., M_GX12] = x1 + x2
    meta[..., M_GY12] = y1 + y2

    g9 = np.empty((B, N, 9, 85), np.float32)
    b_ix = np.arange(B)[:, None, None]
    a_ix = np.arange(A)[None, None, :]
    for s in range(3):
        stride = np.float32(STRIDES[s])
        W = WS[s]
        gx = np.clip((gcx / stride).astype(np.int32), 0, W - 1)
        gy = np.clip((gcy / stride).astype(np.int32), 0, W - 1)
        acx = (gx.astype(np.float32) + np.float32(0.5)) * stride
        acy = (gy.astype(np.float32) + np.float32(0.5)) * stride
        for a in range(A):
            meta[..., M_AC + s * 3 + a] = acx
            meta[..., M_AC + 9 + s * 3 + a] = acy
            aw, ah = ANCHOR_WH[s][a]
            meta[..., M_AWH + s * 3 + a] = aw
            meta[..., M_AWH + 9 + s * 3 + a] = ah
            meta[..., M_ST + s * 3 + a] = stride
            meta[..., M_ST + 9 + s * 3 + a] = stride

        # anchor IoU (f32, mirrors reference order)
        iou = np.empty((B, N, A), np.float32)
        for a in range(A):
            aw = np.float32(ANCHOR_WH[s][a][0])
            ah = np.float32(ANCHOR_WH[s][a][1])
            ax1 = acx - aw * np.float32(0.5)
            ay1 = acy - ah * np.float32(0.5)
            ax2 = acx + aw * np.float32(0.5)
            ay2 = acy + ah * np.float32(0.5)
            iw = np.clip(np.minimum(x2, ax2) - np.maximum(x1, ax1), 0.0, None)
            ih = np.clip(np.minimum(y2, ay2) - np.maximum(y1, ay1), 0.0, None)
            inter = iw * ih
            iou[..., a] = inter / (ag + aw * ah - inter + np.float32(EPS))
        pos = iou > 0.5
        best = np.zeros_like(pos)
        np.put_along_axis(best, np.argmax(iou, -1)[..., None], True, axis=-1)
        posf = np.where(pos.any(-1, keepdims=True), pos, best)

        key = ((b_ix * A + a_ix) * W + gy[:, :, None]) * W + gx[:, :, None]
        flat = B * A * W * W
        cellmax = np.full(flat, -1.0, np.float32)
        np.maximum.at(cellmax, key.ravel(),
                      np.where(posf, iou, np.float32(-1.0)).ravel())
        win = posf & (iou == cellmax[key.ravel()].reshape(B, N, A))
        meta[..., M_WIN + s * 3:M_WIN + (s + 1) * 3] = win.astype(np.float32)

        pred = np.asarray(inputs[f"pred{s}"], np.float32) \
            .reshape(B, A, HWS[s], 85)
        cell = gy * W + gx
        g9[:, :, s * 3:(s + 1) * 3, :] = pred[b_ix, a_ix, cell[:, :, None], :]

    # label-selected cls logit per (entry, slot): a gather, done host-side.
    # Pre-multiplied by win so the device accumulates it directly.
    win9 = meta[..., M_WIN:M_WIN + 9]
    meta[..., M_PL:M_PL + 9] = win9 * np.take_along_axis(
        g9[..., 5:85], lbl[:, :, None, None].repeat(9, axis=2), axis=-1)[..., 0]
    # mask non-winning slots' cls logits to PAD_VAL: softplus(PAD_VAL) == 0,
    # so the device-side softplus accum equals the win-weighted cls sum
    g9[..., 5:85] = np.where(win9[..., None] > 0, g9[..., 5:85],
                             np.float32(PAD_VAL))
    return meta, g9


def _prep_core_inputs(inputs, meta, g9, core):
    b0 = core * BLOC
    ch4 = np.empty((128, 396), np.float32)
    for s in range(3):
        plane = np.full(128 * NCOL[s], PAD_VAL, np.float32)
        pr = np.asarray(inputs[f"pred{s}"][b0:b0 + BLOC], np.float32) \
            .reshape(BLOC, A, HWS[s], 85)[..., 4]          # [2, 3, HW]
        pr = pr.transpose(0, 2, 1).ravel()                  # [b, cell, a]
        plane[:pr.shape[0]] = pr
        ch4[:, CB4[s]:CB4[s] + NCOL[s]] = plane.reshape(128, NCOL[s])
    gc = g9[b0:b0 + BLOC]
    din = np.concatenate(
        [gc[..., 0:5].reshape(NP, 45),
         meta[b0:b0 + BLOC].reshape(NP, MW)], axis=1)
    return {
        "din": np.ascontiguousarray(din),
        "gcls": np.ascontiguousarray(
            gc[..., 5:85].astype(np.float16)).reshape(NP, 9 * 80),
        "ch4": ch4.astype(np.float16),
    }


def _combine(parts):
    """Host-side all-reduce of the 18 per-core accumulators + final
    normalization."""
    acc = np.zeros(13, dtype=np.float64)
    for p in parts:
        acc += p.astype(np.float64)
    box_s = acc[0]
    objp_s = acc[1]
    cls_s = acc[2] - acc[3]
    npos = 0.0
    objn_s = 0.0
    for s in range(3):
        npos_s = acc[4 + s]
        negc = acc[7 + s]
        dsum_s = acc[10 + s]
        npos += npos_s
        flat = B * A * HWS[s]
        num_neg = flat - npos_s
        objn_s += (dsum_s - negc) / max(num_neg, 1.0)
    tp = max(npos, 1.0)
    box_loss = box_s / tp
    obj_pos_loss = objp_s / tp
    obj_neg_loss = objn_s / 3.0
    cls_loss = cls_s / tp
    total = box_loss + obj_pos_loss + obj_neg_loss + cls_loss
    vals = [total, box_loss, obj_pos_loss, obj_neg_loss, cls_loss]
    if not np.isfinite(total):
        vals = [0.0] * 5
    return tuple(np.asarray(v, dtype=np.float32) for v in vals)


def kernel(**inputs):
    inputs.pop("_variant", None)
    trace = inputs.pop("_trace", False)
    if "nc" not in _CACHE:
        _CACHE["nc"] = build_nc()
    nc = _CACHE["nc"]
    meta, g9 = _host_assign(inputs)
    in_maps = [_prep_core_inputs(inputs, meta, g9, c) for c in range(NCORES)]
    res = run_bass_kernel_spmd(nc, in_maps, core_ids=list(range(NCORES)),
                               trace=trace)
    parts = [r["out"].astype(np.float64).sum(axis=0) for r in res.results]
    outv = _combine(parts)
    kernel._last_results = res
    return outv
